# revision 21
# baseline (speedup 1.0000x reference)
"""ContextQueryAttention Trainium2 Bass kernel (v3).

Full-input contract: kernel(context[64,1024,128], query[64,128,128],
W[384,1], query_mask[64,128]) -> out[64,1024,512] (f32).

Sharding: data-parallel over batch B across 8 NeuronCores (8 batches/core).

v3 design (per batch, c = p*8 + t layout):
  - 3-stage software pipeline: stage1(b) | stage2a(b-1) | stage2b(b-2)
    so the latency-heavy q2c scalar chain of b-2 overlaps bulk work.
  - c_term fused into the S matmul rhs (rhs_s = qT*w_s + w_c); rowmax of
    (S + qrow) directly gives M[c] for q2c.
  - S matmuls in 2 waves of 4 tiles; ctxT transposes write the junk half
    of the 256-wide fp32r S psum tile (bufs=2 on a 2-bank wave tile).
  - per-tile exp on ACT with bias=-M and accum_out=Z (denominator free).
  - e/eT/c2q and the whole q2c path in bf16 (1 cycle/row, no fp32r
    even-width ISA rules, bf16 psum tiles = half the banks).
  - out cols 0:128 stored straight from the ctx load tile; cols 128:512
    assembled in a [128,8,384] stage tile, stored as two 6KB chunks.
"""

import sys

import numpy as np

try:
    import concourse.bass as bass  # noqa: F401
except ImportError:  # grading dir may lack the site config
    sys.path.insert(0, "/opt/trn_rl_repo")

import concourse.bass as bass
import concourse.mybir as mybir
import concourse.tile as tile
from concourse import bacc
from concourse.bass_utils import run_bass_kernel_spmd
from concourse.masks import make_identity

F32 = mybir.dt.float32
F32R = mybir.dt.float32r
BF16 = mybir.dt.bfloat16
P = 128          # partitions
D = 128          # feature dim
Q = 128          # query len
C = 1024         # context len
CT = C // P      # context tiles per batch (8)
CW = CT // 2     # tiles per S wave (4)
N_CORES = 8
B_FULL = 64
B_SHARD = B_FULL // N_CORES  # 8 batches per core
W_PAD = 256      # fp32r fast path needs moving/out free dim >= 256


def build_program(n_batches: int = B_SHARD) -> bass.Bass:
    nc = bacc.Bacc(None, target_bir_lowering=False)

    ctx_d = nc.declare_dram_parameter("context", [n_batches, C, D], F32, isOutput=False)
    qry_d = nc.declare_dram_parameter("query", [n_batches, Q, D], F32, isOutput=False)
    w_d = nc.declare_dram_parameter("W", [3 * D, 1], F32, isOutput=False)
    msk_d = nc.declare_dram_parameter("query_mask", [n_batches, Q], F32, isOutput=False)
    out_d = nc.declare_dram_parameter("out", [n_batches, C, 4 * D], F32, isOutput=True)

    with tile.TileContext(nc) as tc:
        with (
            tc.tile_pool(name="singles", bufs=1) as singles,
            tc.tile_pool(name="ctxp", bufs=4) as ctxp,
            tc.tile_pool(name="qryp", bufs=3) as qryp,
            tc.tile_pool(name="bp", bufs=2) as bp,
            tc.tile_pool(name="sp", bufs=3) as sp,
            tc.tile_pool(name="spqp", bufs=2) as spqp,
            tc.tile_pool(name="ep", bufs=2) as ep,
            tc.tile_pool(name="tp", bufs=2) as tp,
            tc.tile_pool(name="cbf", bufs=2) as cbf,
            tc.tile_pool(name="stp", bufs=3) as stp,
            tc.tile_pool(name="ps_s", bufs=2, space="PSUM") as ps_s,
            tc.tile_pool(name="ps_misc", bufs=1, space="PSUM") as ps_misc,
            tc.tile_pool(name="ps_et", bufs=1, space="PSUM") as ps_et,
            tc.tile_pool(name="ps_c2q", bufs=1, space="PSUM") as ps_c2q,
        ):
            # ---- one-time constants ----
            identity_f = singles.tile([P, P], F32)
            make_identity(nc, identity_f)
            identity_bf = singles.tile([P, P], BF16)
            nc.vector.tensor_copy(out=identity_bf, in_=identity_f)
            ones_col = singles.tile([P, 1], F32)
            nc.vector.memset(ones_col, 1.0)
            ones_bf = singles.tile([P, 1], BF16)
            nc.vector.tensor_copy(out=ones_bf, in_=ones_col)

            # W [384,1] -> wvec_f [128,3] (cols: w_c, w_q, w_s)
            w3 = singles.tile([3, P], F32)
            nc.sync.dma_start(out=w3, in_=w_d.rearrange("(g d) o -> g (d o)", g=3))
            wv_ps = ps_misc.tile([P, 512], F32, tag="misc")
            nc.tensor.transpose(wv_ps[:, 0:3], w3, identity_f[:3, :3])
            wvec_f = singles.tile([P, 3], F32)
            nc.scalar.copy(wvec_f, wv_ps[:, 0:3])
            wvec_r = singles.tile([P, 3], F32R)
            nc.vector.tensor_copy(out=wvec_r, in_=wv_ps[:, 0:3])

            # maskterm row: (1-mask)*NEG_INF for all batches
            msk_row = singles.tile([1, n_batches * Q], F32)
            nc.sync.dma_start(out=msk_row, in_=msk_d.rearrange("b q -> (b q)")[None, :])
            maskterm = singles.tile([1, n_batches * Q], F32)
            nc.vector.tensor_scalar(
                maskterm, msk_row, 1e9, -1e9,
                op0=mybir.AluOpType.mult, op1=mybir.AluOpType.add,
            )

            state = {}

            def stage1(b):
                st = {}
                # ---- loads ----
                ctx_sb = ctxp.tile([P, CT, D], F32, tag="ctx")
                nc.sync.dma_start(
                    out=ctx_sb, in_=ctx_d[b].rearrange("(p t) d -> p t d", t=CT)
                )
                qry_sb = qryp.tile([Q, D], F32, tag="qry")
                nc.sync.dma_start(out=qry_sb, in_=qry_d[b])
                out_ap = out_d[b].rearrange("(p t) d -> p t d", t=CT)
                # out cols 0:128 = context, straight from the load tile
                nc.sync.dma_start(out=out_ap[:, :, 0:D], in_=ctx_sb)
                qry_bf = qryp.tile([Q, D], BF16, tag="qrybf")
                nc.vector.tensor_copy(out=qry_bf, in_=qry_sb)

                misc_ps = ps_misc.tile([P, 512], F32, tag="misc")

                # qT = transpose(query): [d, q]
                nc.tensor.transpose(misc_ps[:, 0:128], qry_sb, identity_f)
                # ctxT transposes into the junk halves of the S wave tiles
                s_ps = [
                    ps_s.tile([P, CW, W_PAD], F32, tag="s", name=f"s_ps{w}")
                    for w in range(2)
                ]
                for w in range(2):
                    for i in range(CW):
                        nc.tensor.transpose(
                            s_ps[w][:, i, 128:256],
                            ctx_sb[:, w * CW + i, :], identity_f,
                        )

                qT_sb = bp.tile([P, W_PAD], F32R, tag="qT")
                nc.scalar.copy(qT_sb[:, 0:128], misc_ps[:, 0:128])
                # rhs_s = qT*w_s + w_c  (c_term folded in); right half junk
                rhs_s = bp.tile([P, W_PAD], F32R, tag="rhss")
                nc.vector.tensor_scalar(
                    rhs_s[:, 0:128], qT_sb[:, 0:128],
                    wvec_f[:, 2:3], wvec_f[:, 0:1],
                    op0=mybir.AluOpType.mult, op1=mybir.AluOpType.add,
                )
                # q_term[q] = sum_d qT[d,q]*w_q[d] -> [1, 256] (cols 128: junk)
                nc.tensor.matmul(
                    misc_ps[0:1, 256:512], lhsT=wvec_r[:, 1:2], rhs=qT_sb,
                )
                qrow_sb = bp.tile([1, Q], F32, tag="qrow")
                nc.vector.tensor_add(
                    qrow_sb, misc_ps[0:1, 256:384],
                    maskterm[:, b * Q:(b + 1) * Q],
                )
                qbc = bp.tile([P, Q], F32, tag="qbc")
                nc.gpsimd.partition_broadcast(qbc, qrow_sb)

                # ctxT psum -> sbuf (one ACT copy per wave), then S matmuls
                ctxT_sb = tp.tile([P, CT, D], F32R, tag="ctxT")
                spq = spqp.tile([P, CT, Q], F32, tag="spq")
                negM = sp.tile([P, CT], F32, tag="negM")
                for w in range(2):
                    nc.scalar.copy(
                        ctxT_sb[:, w * CW:(w + 1) * CW, :], s_ps[w][:, :, 128:256]
                    )
                    for i in range(CW):
                        nc.tensor.matmul(
                            s_ps[w][:, i, :],
                            lhsT=ctxT_sb[:, w * CW + i, :],
                            rhs=rhs_s,
                        )
                    # spq = S + qrow (broadcast); negM = -rowmax per tile
                    nc.vector.tensor_add(
                        spq[:, w * CW:(w + 1) * CW, :], s_ps[w][:, :, 0:128],
                        qbc[:, :].unsqueeze(1).broadcast_to((P, CW, Q)),
                    )
                    nc.vector.reduce_max(
                        negM[:, w * CW:(w + 1) * CW],
                        spq[:, w * CW:(w + 1) * CW, :],
                        axis=mybir.AxisListType.X, negate=True,
                    )
                # per-tile exp with bias=-M; accum_out gives Z for free
                e_all = ep.tile([P, CT, Q], BF16, tag="e")
                Z = sp.tile([P, CT], F32, tag="Z")
                for t in range(CT):
                    nc.scalar.activation(
                        e_all[:, t, :], spq[:, t, :],
                        mybir.ActivationFunctionType.Exp,
                        bias=negM[:, t:t + 1], accum_out=Z[:, t:t + 1],
                    )
                r_all = sp.tile([P, CT], F32, tag="r")
                nc.vector.reciprocal(r_all, Z)

                st.update(
                    ctx_sb=ctx_sb, qry_bf=qry_bf, misc_ps=misc_ps,
                    e_all=e_all, negM=negM, r_all=r_all, out_ap=out_ap,
                )
                return st

            def stage2a(b, st):
                ctx_sb = st["ctx_sb"]
                e_all = st["e_all"]
                r_all = st["r_all"]
                qry_bf = st["qry_bf"]

                # eT transposes (bf16) + copy to sbuf
                et_ps = ps_et.tile([P, CT, Q], BF16, tag="et")
                for t in range(CT):
                    nc.tensor.transpose(
                        et_ps[:, t, :], e_all[:, t, :], identity_bf
                    )
                eT_all = tp.tile([P, CT, Q], BF16, tag="eT")
                nc.vector.tensor_copy(out=eT_all, in_=et_ps)

                # c2q matmuls (bf16 in, f32 psum out)
                c2q_ps = ps_c2q.tile([P, CT, D], F32, tag="c2q")
                for t in range(CT):
                    nc.tensor.matmul(
                        c2q_ps[:, t, :], lhsT=eT_all[:, t, :], rhs=qry_bf
                    )

                stage = stp.tile([P, CT, 3 * D], F32, tag="stage")
                # stage cols 0:128 (out 128:256) = c2q_unnorm * r
                for t in range(CT):
                    if t % 8 in (3, 6):
                        nc.vector.tensor_scalar_mul(
                            stage[:, t, 0:D], c2q_ps[:, t, :], r_all[:, t:t + 1],
                        )
                    else:
                        nc.scalar.mul(
                            stage[:, t, 0:D], c2q_ps[:, t, :], r_all[:, t:t + 1],
                        )
                # stage cols 128:256 (out 256:384) = ctx * c2q
                nc.gpsimd.tensor_mul(
                    stage[:, :, D:2 * D], ctx_sb, stage[:, :, 0:D]
                )
                st["stage"] = stage

            def stage2b(b, st):
                ctx_sb = st["ctx_sb"]
                negM = st["negM"]
                misc_ps = st["misc_ps"]
                stage = st["stage"]
                out_ap = st["out_ap"]

                # bf16 ctx for the q2c weighted sum
                ctx_bf = cbf.tile([P, CT, D], BF16, tag="ctxbf")
                nc.vector.tensor_copy(out=ctx_bf, in_=ctx_sb)

                # global max g over the batch: M = -negM
                nm_col = sp.tile([P, 1], F32, tag="nm")
                nc.vector.tensor_reduce(
                    nm_col, negM, axis=mybir.AxisListType.X, op=mybir.AluOpType.min
                )
                nc.tensor.transpose(misc_ps[0:1, 128:256], nm_col, identity_f)
                ng = sp.tile([1, 1], F32, tag="ng")
                nc.vector.tensor_reduce(
                    ng, misc_ps[0:1, 128:256], axis=mybir.AxisListType.X,
                    op=mybir.AluOpType.min,
                )
                ng_col = sp.tile([P, 1], F32, tag="ngc")
                nc.gpsimd.partition_broadcast(ng_col, ng)
                # eM = exp(M - g) = exp(-negM + ng)  (bf16)
                eM_bf = sp.tile([P, CT], BF16, tag="eM")
                nc.scalar.activation(
                    eM_bf, negM, mybir.ActivationFunctionType.Exp,
                    bias=ng_col, scale=-1.0,
                )
                # T = total sum of eM: partition-sum -> [1, CT] in q_term junk
                nc.tensor.matmul(
                    misc_ps[0:1, 384:392], lhsT=ones_bf, rhs=eM_bf,
                )
                Tsum = sp.tile([1, 1], F32, tag="Tsum")
                nc.vector.tensor_reduce(
                    Tsum, misc_ps[0:1, 384:392], axis=mybir.AxisListType.X,
                    op=mybir.AluOpType.add,
                )
                rT = sp.tile([1, 1], F32, tag="rT")
                nc.vector.reciprocal(rT, Tsum)
                # q2cT[d] = sum_t sum_c ctx[c,t,d]*eM[c,t] (psum accumulate)
                for t in range(CT):
                    nc.tensor.matmul(
                        misc_ps[:, 252:253],
                        lhsT=ctx_bf[:, t, :],
                        rhs=eM_bf[:, t:t + 1],
                        start=(t == 0), stop=(t == CT - 1),
                    )
                q2c_col = sp.tile([P, 1], F32, tag="q2ccol")
                nc.vector.tensor_copy(out=q2c_col, in_=misc_ps[:, 252:253])
                nc.tensor.transpose(misc_ps[0:1, 128:256], q2c_col, identity_f)
                q2c_row = bp.tile([1, D], F32, tag="q2crow")
                nc.vector.tensor_scalar_mul(q2c_row, misc_ps[0:1, 128:256], rT)
                q2c_bc = bp.tile([P, D], F32, tag="q2cbc")
                nc.gpsimd.partition_broadcast(q2c_bc, q2c_row)
                # stage cols 256:384 (out 384:512) = ctx * q2c
                nc.gpsimd.tensor_mul(
                    stage[:, :, 2 * D:3 * D], ctx_sb,
                    q2c_bc[:, :].unsqueeze(1).broadcast_to((P, CT, D)),
                )

                # ---- store out cols 128:512 (two 6KB/partition chunks) ----
                nc.sync.dma_start(
                    out=out_ap[:, 0:CW, D:], in_=stage[:, 0:CW, :]
                )
                nc.sync.dma_start(
                    out=out_ap[:, CW:, D:], in_=stage[:, CW:, :]
                )

            for b in range(n_batches + 2):
                if b < n_batches:
                    state[b] = stage1(b)
                if 0 < b <= n_batches:
                    stage2a(b - 1, state[b - 1])
                if b > 1:
                    stage2b(b - 2, state.pop(b - 2))

    nc.compile()
    return nc


_CACHED = {}


def _get_program(n_batches: int = B_SHARD) -> bass.Bass:
    if n_batches not in _CACHED:
        _CACHED[n_batches] = build_program(n_batches)
    return _CACHED[n_batches]


def kernel(context, query, W, query_mask, **run_kwargs):
    context = np.ascontiguousarray(np.asarray(context, dtype=np.float32))
    query = np.ascontiguousarray(np.asarray(query, dtype=np.float32))
    W = np.ascontiguousarray(np.asarray(W, dtype=np.float32))
    query_mask = np.ascontiguousarray(np.asarray(query_mask, dtype=np.float32))

    nc = _get_program(B_SHARD)
    in_maps = []
    for c in range(N_CORES):
        s = slice(c * B_SHARD, (c + 1) * B_SHARD)
        in_maps.append(
            {
                "context": np.ascontiguousarray(context[s]),
                "query": np.ascontiguousarray(query[s]),
                "W": W,
                "query_mask": np.ascontiguousarray(query_mask[s]),
            }
        )
    res = run_bass_kernel_spmd(nc, in_maps, core_ids=list(range(N_CORES)), **run_kwargs)
    out = np.concatenate([r["out"] for r in res.results], axis=0)
    if run_kwargs:
        kernel.last_result = res
    return out


# revision 23
# speedup vs baseline: 1.6032x; 1.6032x over previous
"""ContextQueryAttention Trainium2 Bass kernel (v4).

Full-input contract: kernel(context[64,1024,128], query[64,128,128],
W[384,1], query_mask[64,128]) -> out[64,1024,512] (f32).

Sharding: data-parallel over batch B across 8 NeuronCores (8 batches/core).

v4 design (per batch, c = p*8 + t layout):
  - All PE matmuls/transposes in bf16 (1 cycle/row at any width; no
    fp32r 256-wide padding, no fp32r even-width ISA rules). Verified
    numerically: rel err ~7e-3 vs the 2e-2 gate.
  - c_term fused into the S matmul rhs (rhs_s = qT*w_s + w_c); rowmax of
    (S + qrow) directly gives M[c] for the q2c path.
  - qrow broadcast via an f32r ones-matmul into PSUM (read in place by
    the DVE softmax add); g and q2c broadcasts in bf16 (g is a global
    offset that cancels; the q2c path is bf16 anyway).
  - softmax: one big DVE add / rowmax / sub chain + ONE 1024-wide exp on
    ACT (accum_out avoided: it costs a separate 280ns readout op).
  - 3-stage software pipeline: stage1(b) | stage2a(b-1) | stage2b(b-2)
    so the latency-heavy q2c scalar chain overlaps bulk work two batches
    back.
  - out cols 0:128 stored straight from the ctx load tile; cols 128:512
    assembled in a [128,8,384] stage tile, stored as two 6KB chunks.
"""

import sys

import numpy as np

try:
    import concourse.bass as bass  # noqa: F401
except ImportError:  # grading dir may lack the site config
    sys.path.insert(0, "/opt/trn_rl_repo")

import concourse.bass as bass
import concourse.mybir as mybir
import concourse.tile as tile
from concourse import bacc
from concourse.bass_utils import run_bass_kernel_spmd
from concourse.masks import make_identity

F32 = mybir.dt.float32
F32R = mybir.dt.float32r
BF16 = mybir.dt.bfloat16
P = 128          # partitions
D = 128          # feature dim
Q = 128          # query len
C = 1024         # context len
CT = C // P      # context tiles per batch (8)
N_CORES = 8
B_FULL = 64
B_SHARD = B_FULL // N_CORES  # 8 batches per core


def build_program(n_batches: int = B_SHARD) -> bass.Bass:
    nc = bacc.Bacc(None, target_bir_lowering=False)

    ctx_d = nc.declare_dram_parameter("context", [n_batches, C, D], F32, isOutput=False)
    qry_d = nc.declare_dram_parameter("query", [n_batches, Q, D], F32, isOutput=False)
    w_d = nc.declare_dram_parameter("W", [3 * D, 1], F32, isOutput=False)
    msk_d = nc.declare_dram_parameter("query_mask", [n_batches, Q], F32, isOutput=False)
    out_d = nc.declare_dram_parameter("out", [n_batches, C, 4 * D], F32, isOutput=True)

    with tile.TileContext(nc) as tc:
        with (
            tc.tile_pool(name="singles", bufs=1) as singles,
            tc.tile_pool(name="ctxp", bufs=4) as ctxp,
            tc.tile_pool(name="cbfp", bufs=4) as cbfp,
            tc.tile_pool(name="qryp", bufs=3) as qryp,
            tc.tile_pool(name="bp", bufs=2) as bp,
            tc.tile_pool(name="sp", bufs=3) as sp,
            tc.tile_pool(name="spqp", bufs=2) as spqp,
            tc.tile_pool(name="ep", bufs=2) as ep,
            tc.tile_pool(name="tp", bufs=2) as tp,
            tc.tile_pool(name="stp", bufs=3) as stp,
            tc.tile_pool(name="ps_qt", bufs=1, space="PSUM") as ps_qt,
            tc.tile_pool(name="ps_ct", bufs=1, space="PSUM") as ps_ct,
            tc.tile_pool(name="ps_s", bufs=1, space="PSUM") as ps_s,
            tc.tile_pool(name="ps_misc", bufs=1, space="PSUM") as ps_misc,
            tc.tile_pool(name="ps_et", bufs=1, space="PSUM") as ps_et,
            tc.tile_pool(name="ps_c2q", bufs=1, space="PSUM") as ps_c2q,
        ):
            # ---- one-time constants ----
            identity_f = singles.tile([P, P], F32)
            make_identity(nc, identity_f)
            identity_bf = singles.tile([P, P], BF16)
            nc.vector.tensor_copy(out=identity_bf, in_=identity_f)
            ones_colbf = singles.tile([P, 1], BF16)
            nc.vector.memset(ones_colbf, 1.0)
            ones_rowbf = singles.tile([1, P], BF16)
            nc.vector.memset(ones_rowbf, 1.0)
            ones_rowf = singles.tile([1, P], F32)
            nc.vector.memset(ones_rowf, 1.0)
            ones_rowr = singles.tile([1, P], F32R)
            nc.vector.tensor_copy(out=ones_rowr, in_=ones_rowf)

            # W [384,1] -> wvec [128,3] (cols: w_c, w_q, w_s)
            w3 = singles.tile([3, P], F32)
            nc.sync.dma_start(out=w3, in_=w_d.rearrange("(g d) o -> g (d o)", g=3))
            wv_ps = ps_misc.tile([P, 512], F32, tag="misc")
            nc.tensor.transpose(wv_ps[:, 0:3], w3, identity_f[:3, :3])
            wvec_f = singles.tile([P, 3], F32)
            nc.scalar.copy(wvec_f, wv_ps[:, 0:3])
            wvec_bf = singles.tile([P, 3], BF16)
            nc.vector.tensor_copy(out=wvec_bf, in_=wv_ps[:, 0:3])

            # maskterm row: (1-mask)*NEG_INF for all batches
            msk_row = singles.tile([1, n_batches * Q], F32)
            nc.sync.dma_start(out=msk_row, in_=msk_d.rearrange("b q -> (b q)")[None, :])
            maskterm = singles.tile([1, n_batches * Q], F32)
            nc.vector.tensor_scalar(
                maskterm, msk_row, 1e9, -1e9,
                op0=mybir.AluOpType.mult, op1=mybir.AluOpType.add,
            )

            state = {}

            def stage1(b):
                st = {}
                # ---- loads ----
                ctx_sb = ctxp.tile([P, CT, D], F32, tag="ctx")
                nc.sync.dma_start(
                    out=ctx_sb, in_=ctx_d[b].rearrange("(p t) d -> p t d", t=CT)
                )
                qry_sb = qryp.tile([Q, D], F32, tag="qry")
                nc.sync.dma_start(out=qry_sb, in_=qry_d[b])
                out_ap = out_d[b].rearrange("(p t) d -> p t d", t=CT)
                # out cols 0:128 = context, straight from the load tile
                nc.sync.dma_start(out=out_ap[:, :, 0:D], in_=ctx_sb)
                qry_bf = qryp.tile([Q, D], BF16, tag="qrybf")
                nc.vector.tensor_copy(out=qry_bf, in_=qry_sb)
                ctx_bf = cbfp.tile([P, CT, D], BF16, tag="ctxbf")
                nc.vector.tensor_copy(out=ctx_bf, in_=ctx_sb)

                misc_ps = ps_misc.tile([P, 512], F32, tag="misc")

                # qT = transpose(query) in bf16: [d, q]
                qt_ps = ps_qt.tile([P, Q], BF16, tag="qt")
                nc.tensor.transpose(qt_ps, qry_bf, identity_bf)
                # ctxT transposes (bf16)
                ct_ps = ps_ct.tile([P, CT, D], BF16, tag="ct")
                for t in range(CT):
                    nc.tensor.transpose(
                        ct_ps[:, t, :], ctx_bf[:, t, :], identity_bf
                    )

                qT_sb = bp.tile([P, Q], BF16, tag="qT")
                nc.scalar.copy(qT_sb, qt_ps)
                # rhs_s = qT*w_s + w_c  (c_term folded in)
                rhs_s = bp.tile([P, Q], BF16, tag="rhss")
                nc.vector.tensor_scalar(
                    rhs_s, qT_sb, wvec_f[:, 2:3], wvec_f[:, 0:1],
                    op0=mybir.AluOpType.mult, op1=mybir.AluOpType.add,
                )
                ctxT_sb = tp.tile([P, CT, D], BF16, tag="ctxT")
                nc.scalar.copy(ctxT_sb, ct_ps)

                # q_term[q] = sum_d qT[d,q]*w_q[d] -> [1, 128]
                nc.tensor.matmul(
                    misc_ps[0:1, 256:384], lhsT=wvec_bf[:, 1:2], rhs=qT_sb,
                )
                # S matmuls (bf16, 128-wide)
                s_ps = ps_s.tile([P, CT, Q], F32, tag="s")
                for t in range(CT):
                    nc.tensor.matmul(
                        s_ps[:, t, :], lhsT=ctxT_sb[:, t, :], rhs=rhs_s,
                    )
                # qrow = q_term + maskterm (f32r for the exact broadcast)
                qrow_r = bp.tile([1, Q], F32R, tag="qrow")
                nc.vector.tensor_add(
                    qrow_r, misc_ps[0:1, 256:384],
                    maskterm[:, b * Q:(b + 1) * Q],
                )
                # qbc = broadcast of qrow to all partitions (overwrites the
                # consumed q_term region; read in place from psum by DVE)
                nc.tensor.matmul(
                    misc_ps[:, 256:384], lhsT=ones_rowr, rhs=qrow_r,
                )
                qbc_sb = bp.tile([P, Q], F32, tag="qbc")
                nc.scalar.copy(qbc_sb, misc_ps[:, 256:384])

                # softmax: spq = S + qrow; negM = -rowmax; spq -= M; exp
                spq = spqp.tile([P, CT, Q], F32, tag="spq")
                nc.vector.tensor_add(
                    spq, s_ps[:, :, :],
                    qbc_sb[:, :].unsqueeze(1).broadcast_to((P, CT, Q)),
                )
                negM = sp.tile([P, CT], F32, tag="negM")
                nc.vector.reduce_max(
                    negM, spq, axis=mybir.AxisListType.X, negate=True
                )
                spq2 = spqp.tile([P, CT, Q], F32, tag="spq2")
                nc.vector.tensor_add(
                    spq2, spq,
                    negM[:, :].unsqueeze(2).broadcast_to((P, CT, Q)),
                )
                e_all = ep.tile([P, CT, Q], BF16, tag="e")
                nc.scalar.activation(
                    e_all, spq2, mybir.ActivationFunctionType.Exp
                )
                Z = sp.tile([P, CT], F32, tag="Z")
                nc.vector.tensor_reduce(
                    Z, e_all, axis=mybir.AxisListType.X, op=mybir.AluOpType.add
                )
                r_all = sp.tile([P, CT], F32, tag="r")
                nc.vector.reciprocal(r_all, Z)

                st.update(
                    ctx_sb=ctx_sb, ctx_bf=ctx_bf, qry_bf=qry_bf,
                    misc_ps=misc_ps, e_all=e_all, negM=negM, r_all=r_all,
                    out_ap=out_ap,
                )
                return st

            def stage2a(b, st):
                ctx_sb = st["ctx_sb"]
                e_all = st["e_all"]
                r_all = st["r_all"]
                qry_bf = st["qry_bf"]

                # eT transposes (bf16) + copy to sbuf
                et_ps = ps_et.tile([P, CT, Q], BF16, tag="et")
                for t in range(CT):
                    nc.tensor.transpose(
                        et_ps[:, t, :], e_all[:, t, :], identity_bf
                    )
                eT_all = tp.tile([P, CT, Q], BF16, tag="eT")
                nc.scalar.copy(eT_all, et_ps)

                # c2q matmuls (bf16 in, f32 psum out)
                c2q_ps = ps_c2q.tile([P, CT, D], F32, tag="c2q")
                for t in range(CT):
                    nc.tensor.matmul(
                        c2q_ps[:, t, :], lhsT=eT_all[:, t, :], rhs=qry_bf
                    )

                stage = stp.tile([P, CT, 3 * D], F32, tag="stage")
                # stage cols 0:128 (out 128:256) = c2q_unnorm * r
                for t in range(CT):
                    if t in (2, 5):
                        nc.vector.tensor_scalar_mul(
                            stage[:, t, 0:D], c2q_ps[:, t, :], r_all[:, t:t + 1],
                        )
                    else:
                        nc.scalar.mul(
                            stage[:, t, 0:D], c2q_ps[:, t, :], r_all[:, t:t + 1],
                        )
                # stage cols 128:256 (out 256:384) = ctx * c2q
                nc.gpsimd.tensor_mul(
                    stage[:, :, D:2 * D], ctx_sb, stage[:, :, 0:D]
                )
                st["stage"] = stage

            def stage2b(b, st):
                ctx_sb = st["ctx_sb"]
                ctx_bf = st["ctx_bf"]
                negM = st["negM"]
                misc_ps = st["misc_ps"]
                stage = st["stage"]
                out_ap = st["out_ap"]

                # global max g over the batch: M = -negM
                nm_col = sp.tile([P, 1], F32, tag="nm")
                nc.vector.tensor_reduce(
                    nm_col, negM, axis=mybir.AxisListType.X, op=mybir.AluOpType.min
                )
                nc.tensor.transpose(misc_ps[0:1, 128:256], nm_col, identity_f)
                ng = sp.tile([1, 1], F32, tag="ng")
                nc.vector.tensor_reduce(
                    ng, misc_ps[0:1, 128:256], axis=mybir.AxisListType.X,
                    op=mybir.AluOpType.min,
                )
                # broadcast ng to a [128,1] column via bf16 ones-matmul (g is
                # a global offset -> bf16 rounding cancels in the softmax)
                ng_bf = sp.tile([1, 1], BF16, tag="ngbf")
                nc.vector.tensor_copy(out=ng_bf, in_=ng)
                nc.tensor.matmul(
                    misc_ps[:, 0:1], lhsT=ones_rowbf, rhs=ng_bf,
                )
                ng_col = sp.tile([P, 1], F32, tag="ngc")
                nc.scalar.copy(ng_col, misc_ps[:, 0:1])
                # eM = exp(M - g) = exp(-negM + ng)  (bf16)
                eM_bf = sp.tile([P, CT], BF16, tag="eM")
                nc.scalar.activation(
                    eM_bf, negM, mybir.ActivationFunctionType.Exp,
                    bias=ng_col, scale=-1.0,
                )
                # T = total sum of eM: partition-sum -> [1, CT] in junk space
                nc.tensor.matmul(
                    misc_ps[0:1, 384:392], lhsT=ones_colbf, rhs=eM_bf,
                )
                Tsum = sp.tile([1, 1], F32, tag="Tsum")
                nc.vector.tensor_reduce(
                    Tsum, misc_ps[0:1, 384:392], axis=mybir.AxisListType.X,
                    op=mybir.AluOpType.add,
                )
                rT = sp.tile([1, 1], F32, tag="rT")
                nc.vector.reciprocal(rT, Tsum)
                # q2cT[d] = sum_t sum_c ctx[c,t,d]*eM[c,t] (psum accumulate)
                for t in range(CT):
                    nc.tensor.matmul(
                        misc_ps[:, 4:5],
                        lhsT=ctx_bf[:, t, :],
                        rhs=eM_bf[:, t:t + 1],
                        start=(t == 0), stop=(t == CT - 1),
                    )
                q2c_col = sp.tile([P, 1], F32, tag="q2ccol")
                nc.vector.tensor_copy(out=q2c_col, in_=misc_ps[:, 4:5])
                nc.tensor.transpose(misc_ps[0:1, 128:256], q2c_col, identity_f)
                q2c_row = bp.tile([1, D], F32, tag="q2crow")
                nc.vector.tensor_scalar_mul(q2c_row, misc_ps[0:1, 128:256], rT)
                q2c_bc = bp.tile([P, D], F32, tag="q2cbc")
                nc.gpsimd.partition_broadcast(q2c_bc, q2c_row)
                # stage cols 256:384 (out 384:512) = ctx * q2c
                nc.gpsimd.tensor_mul(
                    stage[:, :, 2 * D:3 * D], ctx_sb,
                    q2c_bc[:, :].unsqueeze(1).broadcast_to((P, CT, D)),
                )

                # ---- store out cols 128:512 (two 6KB/partition chunks) ----
                nc.sync.dma_start(
                    out=out_ap[:, 0:CT // 2, D:], in_=stage[:, 0:CT // 2, :]
                )
                nc.sync.dma_start(
                    out=out_ap[:, CT // 2:, D:], in_=stage[:, CT // 2:, :]
                )

            for b in range(n_batches + 2):
                if b < n_batches:
                    state[b] = stage1(b)
                if 0 < b <= n_batches:
                    stage2a(b - 1, state[b - 1])
                if b > 1:
                    stage2b(b - 2, state.pop(b - 2))

    nc.compile()
    return nc


_CACHED = {}


def _get_program(n_batches: int = B_SHARD) -> bass.Bass:
    if n_batches not in _CACHED:
        _CACHED[n_batches] = build_program(n_batches)
    return _CACHED[n_batches]


def kernel(context, query, W, query_mask, **run_kwargs):
    context = np.ascontiguousarray(np.asarray(context, dtype=np.float32))
    query = np.ascontiguousarray(np.asarray(query, dtype=np.float32))
    W = np.ascontiguousarray(np.asarray(W, dtype=np.float32))
    query_mask = np.ascontiguousarray(np.asarray(query_mask, dtype=np.float32))

    nc = _get_program(B_SHARD)
    in_maps = []
    for c in range(N_CORES):
        s = slice(c * B_SHARD, (c + 1) * B_SHARD)
        in_maps.append(
            {
                "context": np.ascontiguousarray(context[s]),
                "query": np.ascontiguousarray(query[s]),
                "W": W,
                "query_mask": np.ascontiguousarray(query_mask[s]),
            }
        )
    res = run_bass_kernel_spmd(nc, in_maps, core_ids=list(range(N_CORES)), **run_kwargs)
    out = np.concatenate([r["out"] for r in res.results], axis=0)
    if run_kwargs:
        kernel.last_result = res
    return out
